# revision 11
# baseline (speedup 1.0000x reference)
"""BestBuddyLoss Trainium2 kernel v3 (8-core data parallel).

Per image: q = [unfold(gt) | unfold(down2(gt)) | unfold(down4(gt))] padded to
3072 cols; argmin_j score(i,j) == argmax_j <p1_i+p2_i, q_j> - |q_j|^2, done as
K=28 f32r matmuls (27 data rows + bias row 27).  Logical column j = 2k + b:
even columns stream through stride-2 matmuls into PSUM (3 banks flat,
[128,1536]); odd columns land in transit PSUM banks and are copied to SBUF by
ACT.  One 2-stream custom DVE op per i-tile scans (A_k, B_k) pairs and emits
argmax j = 2*Idx + (A<B) via running-max records.  Tail: PMT permute ->
wrapped i16 idx -> ap_gather q[j*] -> fused |p1 - q*| sum on DVE (custom op,
accum=ADD), pipelined in half-image chunks under the other image's main loop.

Schedule: image-0 prep split across DVE (gt shuffles, adds) / ACT (qsq, bias
copies, downsample copies) / Pool (x shuffle, memsets) / PE, first scan at
~14us.  Image-1 prep runs on Pool + DVE slack interleaved into main(0)'s
emission; its bias matmuls borrow psB slots.
"""

import sys

sys.path.insert(0, "/opt/trn_rl_repo")

import numpy as np

import concourse.bacc as bacc
import concourse.mybir as mybir
import concourse.tile as tile
from concourse.bass_utils import run_bass_kernel_spmd

# ---------------- problem constants (hardcoded) ----------------
B_FULL = 16
NCORES = 8
B_LOC = B_FULL // NCORES       # images per core
C, H, W = 3, 144, 144
G = 48                         # patch grid (144/3)
NI = G * G                     # 2304 query patches
D = 27                         # C*3*3
NQ = NI + (G // 2) ** 2 + (G // 4) ** 2  # 3024
NQP = 3072                     # padded j space (6*512)
KD = 33                        # contraction: 27 data + 5 zero + bias row 32
KZ = 32                        # bias row partition (32-aligned for engine APs)
RG = 32                        # gather channels
IT = 128
NIT = NI // IT                 # 18
HK = NQP // 2                  # 1536 pair count per tile
CH = 512
NCH = HK // CH                 # 3 chunks per half
HNI = NI // 2                  # 1152 per tail half
HT = NIT // 2                  # 9 tiles per tail half
PADBIAS = -1.0e30
CUBIC_W = np.array([-0.09375, 0.59375, 0.59375, -0.09375], dtype=np.float32)

F32 = mybir.dt.float32
F32R = mybir.dt.float32r
I16 = mybir.dt.int16
ADD = mybir.AluOpType.add
SUB = mybir.AluOpType.subtract
MUL = mybir.AluOpType.mult
SQ = mybir.ActivationFunctionType.Square

# ---------------- custom DVE ops ------------------
from concourse.dve_spec import (
    Spec, Src0, Src1, C2, Zero, scan, AluOp, maxx, lower,
)
from concourse.dve_uop import DveOpSpec
import concourse.dve_ops as dve_ops
from concourse.dve_ops import DveOp


def _pair_argmax_ref(in0, in1, c0, c1, c2):
    m = np.maximum(in0, in1)
    run = np.maximum.accumulate(m, axis=-1)
    n = in0.shape[-1]
    j = np.float32(c2) * np.arange(n, dtype=np.float32)[None, :] + (in0 < in1)
    out = (m >= run) * j
    acc = out.reshape(out.shape[0], -1).max(axis=-1, keepdims=True)
    return out.astype(np.float32), acc.astype(np.float32)


def _absdiff_acc_ref(in0, in1, c0, c1, c2):
    out = np.abs(in0 - in1).astype(np.float32)
    acc = out.reshape(out.shape[0], -1).sum(axis=-1, keepdims=True)
    return out, acc.astype(np.float32)


def _register_op(name, spec):
    if name in dve_ops._SUB_OPCODE_FOR_NAME:
        return next(op for op in dve_ops.OPS if op.name == name)
    opcode = dve_ops._CUSTOM_DVE_ROW_BASE + len(dve_ops.OPS)
    shas = {v: DveOpSpec(name=name, opcode=opcode, uops=lower(spec, ver=v),
                         rd1_en=True).sha(v) for v in ("v3", "v4")}
    op = DveOp(name, spec, subdim=False, uops_sha=shas)
    dve_ops.OPS.append(op)
    dve_ops._SUB_OPCODE_FOR_NAME[name] = opcode
    dve_ops.CUSTOM_DVE_SPECS[name] = spec
    return op


def _make_pair_op():
    m = maxx(Src0, Src1)
    two_idx = scan(AluOp.ADD, C2, init=Zero - C2)   # 0, 2, 4, ... (C2=2.0)
    j = two_idx + (Src0 < Src1)
    body = (m >= scan(AluOp.MAX, m)) * j
    return _register_op(
        "ANT_PAIR_ARGMAX",
        Spec(body=body, accum=maxx, reference=_pair_argmax_ref))


def _make_absdiff_op():
    body = maxx(Src0 - Src1, Src1 - Src0)
    return _register_op(
        "ANT_ABSDIFF_ACC",
        Spec(body=body, accum=AluOp.ADD, reference=_absdiff_acc_ref))


PAIR_OP = _make_pair_op()
ABS_OP = _make_absdiff_op()

# ---------------- host-side constants ---------------------------------


def _down_matrix(n, f):
    """M[h, i]: out[i] = sum_h M[h, i] * in[h]  (torch bicubic, offset t=.5)."""
    out_n = n // f
    M = np.zeros((n, out_n), dtype=np.float32)
    for i in range(out_n):
        base = f * i + (f // 2 - 1)
        for a in range(4):
            h = min(max(base + a - 1, 0), n - 1)
            M[h, i] += CUBIC_W[a]
    return M


def _perm_matrices():
    """PMT[:, m*128 + r]: one-hot at row (m*16 + r%16) -> out_m = Pm @ v."""
    P = np.zeros((128, 8 * 128), dtype=np.float32)
    for m in range(8):
        for r in range(128):
            P[m * 16 + r % 16, m * 128 + r] = 1.0
    return P


def make_consts():
    return {
        "cd2": np.ascontiguousarray(_down_matrix(H, 2)),  # [144, 72]
        "cd4": np.ascontiguousarray(_down_matrix(H, 4)),  # [144, 36]
        "idn": np.eye(128, dtype=np.float32),
        "pmt": _perm_matrices(),
        "neg1": np.full((D, 1), -1.0, dtype=np.float32),
        "padq": _padq(),
        "padl": _padl(),
    }


def _padq():
    p = np.zeros((KD - D, NQP), dtype=np.float32)
    p[KZ - D, NQ:NQP] = PADBIAS
    return p


def _padl():
    p = np.zeros((KD - D, NI), dtype=np.float32)
    p[KZ - D, :] = 1.0
    return p


# ---------------- kernel construction ---------------------------------


def build_nc(debug=False):
    nc = bacc.Bacc("TRN2", target_bir_lowering=False)

    x_d = nc.dram_tensor("x", [B_LOC, C, H, W], F32, kind="ExternalInput")
    gt_d = nc.dram_tensor("gt", [B_LOC, C, H, W], F32, kind="ExternalInput")
    cd2_d = nc.dram_tensor("cd2", [H, 72], F32, kind="ExternalInput")
    cd4_d = nc.dram_tensor("cd4", [H, 36], F32, kind="ExternalInput")
    idn_d = nc.dram_tensor("idn", [128, 128], F32, kind="ExternalInput")
    pmt_d = nc.dram_tensor("pmt", [128, 8 * 128], F32, kind="ExternalInput")
    neg1_d = nc.dram_tensor("neg1", [D, 1], F32, kind="ExternalInput")
    padq_d = nc.dram_tensor("padq", [KD - D, NQP], F32R, kind="ExternalInput")
    padl_d = nc.dram_tensor("padl", [KD - D, NI], F32R, kind="ExternalInput")
    d2_d = nc.dram_tensor("scr_d2", [B_LOC, C, 72, 72], F32, kind="Internal")
    d4_d = nc.dram_tensor("scr_d4", [B_LOC, C, 36, 36], F32, kind="Internal")
    loss_d = nc.dram_tensor("loss", [D, 4], F32, kind="ExternalOutput")
    dbg = {}
    if debug:
        dbg["rr"] = nc.dram_tensor("dbg_rr", [B_LOC, KD, NQP], F32, kind="ExternalOutput")
        dbg["lr"] = nc.dram_tensor("dbg_lr", [B_LOC, KD, NI], F32, kind="ExternalOutput")
        dbg["p1t"] = nc.dram_tensor("dbg_p1t", [B_LOC, D, NI], F32, kind="ExternalOutput")
        dbg["idxf"] = nc.dram_tensor("dbg_idxf", [B_LOC, 128, NIT], F32, kind="ExternalOutput")
        dbg["sel"] = nc.dram_tensor("dbg_sel", [B_LOC, 32, NI], F32, kind="ExternalOutput")

    with tile.TileContext(nc) as tc:
        with (
            tc.tile_pool(name="consts", bufs=1) as cpool,
            tc.tile_pool(name="stageA", bufs=2) as apool,
            tc.tile_pool(name="stageB", bufs=2) as bpool,
            tc.tile_pool(name="stageD", bufs=1) as dapool,
            tc.tile_pool(name="prep", bufs=2) as ppool,
            tc.tile_pool(name="persist", bufs=2) as spool,
            tc.tile_pool(name="scoreB", bufs=2) as scpool,
            tc.tile_pool(name="small", bufs=2) as smpool,
            tc.tile_pool(name="psA", bufs=2, space="PSUM") as psA,
            tc.tile_pool(name="psB", bufs=2, space="PSUM") as psB,
        ):
            # ---------- consts ----------
            cd2a_f = cpool.tile([128, 72], F32, tag="cd2af")
            cd2b_f = cpool.tile([16, 72], F32, tag="cd2bf")
            cd4a_f = cpool.tile([128, 36], F32, tag="cd4af")
            cd4b_f = cpool.tile([16, 36], F32, tag="cd4bf")
            cd2a = cpool.tile([128, 72], F32R, tag="cd2a")
            cd2b = cpool.tile([16, 72], F32R, tag="cd2b")
            cd4a = cpool.tile([128, 36], F32R, tag="cd4a")
            cd4b = cpool.tile([16, 36], F32R, tag="cd4b")
            idn_t = cpool.tile([128, 128], F32, tag="idn")
            pmt_t = cpool.tile([128, 8 * 128], F32, tag="pmt")
            neg1_t = cpool.tile([D, 1], F32, tag="neg1")
            neg1r = cpool.tile([D, 1], F32R, tag="neg1r")
            nc.sync.dma_start(cd2a_f[:], cd2_d[0:128, :])
            nc.sync.dma_start(cd2b_f[:], cd2_d[128:144, :])
            nc.sync.dma_start(cd4a_f[:], cd4_d[0:128, :])
            nc.sync.dma_start(cd4b_f[:], cd4_d[128:144, :])
            nc.sync.dma_start(idn_t[:], idn_d[:])
            nc.sync.dma_start(pmt_t[:], pmt_d[:])
            nc.sync.dma_start(neg1_t[:], neg1_d[:])
            nc.vector.tensor_copy(cd2a[:], cd2a_f[:])
            nc.vector.tensor_copy(cd2b[:], cd2b_f[:])
            nc.vector.tensor_copy(cd4a[:], cd4a_f[:])
            nc.vector.tensor_copy(cd4b[:], cd4b_f[:])
            nc.vector.tensor_copy(neg1r[:], neg1_t[:])

            junk = cpool.tile([128, HK], F32, tag="junk")
            part = cpool.tile([D, 4], F32, tag="part")

            # ---------- persistent per-image tiles (bufs=2) ----------
            rrs, lrs, p1ts, idxf, widxs, sels = ({} for _ in range(6))

            def mk_image_tiles(b):
                rrs[b] = spool.tile([KD, NQP], F32R, tag="rr", name=f"rr{b}")
                lrs[b] = spool.tile([KD, NI], F32R, tag="lr", name=f"lr{b}")
                p1ts[b] = spool.tile([D, NI], F32, tag="p1t", name=f"p1t{b}")
                idxf[b] = smpool.tile([128, NIT], F32, tag="idxf",
                                      name=f"idxf{b}")
                widxs[b] = smpool.tile([128, 8 * NIT], I16, tag="widx",
                                       name=f"widx{b}")
                sels[b] = smpool.tile([RG, NI], F32, tag="sel", bufs=1,
                                      name=f"sel{b}")

            def pad_init(b):
                rr, lr = rrs[b], lrs[b]
                # pad columns of the data rows (32-aligned partition start)
                nc.gpsimd.memset(rr[0:KZ, NQ:NQP].bitcast(F32), 0.0)
                # rows 27:33: zeros + bias/ones rows via one DMA each
                nc.gpsimd.dma_start(rr[D:KD, :], padq_d[:])
                nc.gpsimd.dma_start(lr[D:KD, :], padl_d[:])

            # ---------- unfold helpers ----------
            def shuffle(eng, dst_v, src_v):
                if eng is nc.scalar:
                    eng.copy(dst_v, src_v)
                else:
                    eng.tensor_copy(dst_v, src_v)

            def unfold_half(td, b, hf, dst_ap, eng, round_f32r, nm,
                            dma_eng):
                """one half of dram [C,144,144] -> dst [27, 1152]."""
                gh = G // 2
                szh = 3 * gh * G
                at = apool.tile([9, szh], F32, tag="A", name=f"at_{nm}{hf}")
                for c in range(C):
                    src = td[b, c].rearrange(
                        "(gi r) w -> r gi w", r=3)[:, hf * gh:(hf + 1) * gh]
                    nc.sync.dma_start(at[3 * c:3 * c + 3, :], src)
                bt = bpool.tile([9, szh], F32R if round_f32r else F32,
                                tag="B", name=f"bt_{nm}{hf}")
                rearr = at[:, :].rearrange(
                    "p (gi gj s) -> p s gi gj", gi=gh, gj=G, s=3)
                bt_v = bt[:, :].rearrange(
                    "p (s gi gj) -> p s gi gj", s=3, gi=gh)
                shuffle(eng, bt_v, rearr)
                if round_f32r:
                    dma_eng.dma_start(dst_ap.bitcast(F32),
                                      bt[:, :].bitcast(F32))
                else:
                    dma_eng.dma_start(dst_ap, bt[:, :])

            def unfold_small(dsc_d, b, f, dst_ap, eng, nm, dma_eng):
                """downsampled DRAM scratch [C, n, n] -> dst [27, gs*gs]."""
                n = H // f
                gs = n // 3
                sz = 3 * gs * gs
                dat = dapool.tile([9, 3 * 24 * 24], F32, tag="DA",
                                  name=f"dat_{nm}")
                for c in range(C):
                    src = dsc_d[b, c].rearrange("(gi r) w -> r gi w", r=3)
                    nc.sync.dma_start(
                        dat[3 * c:3 * c + 3, 0:gs * n].rearrange(
                            "p (gi w) -> p gi w", gi=gs), src)
                bt = bpool.tile([9, 3 * (G // 2) * G], F32R, tag="B",
                                name=f"bt_{nm}")
                rearr = dat[:, 0:sz].rearrange(
                    "p (gi gj s) -> p s gi gj", gi=gs, gj=gs, s=3)
                bt_v = bt[:, 0:sz].rearrange(
                    "p (s gi gj) -> p s gi gj", s=3, gi=gs)
                shuffle(eng, bt_v, rearr)
                dma_eng.dma_start(dst_ap.bitcast(F32),
                                  bt[:, 0:sz].bitcast(F32))

            # ---------- downsample (all f32r) ----------
            def downsample(b, f, gar, gbr, out_dram, ceng, dma_eng):
                """gt[b] (f32r [128/16, C*H]) --bicubic/f--> out_dram."""
                n = H // f
                cda = cd2a if f == 2 else cd4a
                cdb = cd2b if f == 2 else cd4b

                cnt = [0]

                def ptile():
                    cnt[0] += 1
                    return psB.tile([128, CH], F32, tag="psB",
                                    name=f"psds{b}_{f}_{cnt[0]}")

                def pcopy(dst, srcp):
                    if ceng is nc.scalar:
                        ceng.copy(dst, srcp)
                    else:
                        ceng.tensor_copy(dst, srcp)

                ghp = ptile()
                nc.tensor.matmul(ghp[0:n, 0:C * H], cda[:, 0:n], gar[:],
                                 start=True, stop=False)
                nc.tensor.matmul(ghp[0:n, 0:C * H], cdb[:, 0:n], gbr[:],
                                 start=False, stop=True)
                gh = ppool.tile([72, C * H], F32, tag="gh", bufs=1,
                                name=f"gh{b}_{f}")
                pcopy(gh[0:n, :], ghp[0:n, 0:C * H])
                gh3 = gh[:].rearrange("i (c w) -> i c w", c=C)
                ghta = ppool.tile([128, C * 72], F32R, tag="ghta",
                                  name=f"ghta{b}_{f}")
                ghtb = ppool.tile([16, C * 72], F32R, tag="ghtb",
                                  name=f"ghtb{b}_{f}")
                ghta3 = ghta[:].rearrange("w (c i) -> w c i", c=C)
                ghtb3 = ghtb[:].rearrange("w (c i) -> w c i", c=C)
                for c in range(C):
                    tp = ptile()
                    nc.tensor.transpose(tp[0:128, 0:n], gh3[0:n, c, 0:128],
                                        idn_t[0:n, 0:n])
                    pcopy(ghta3[:, c, 0:n], tp[0:128, 0:n])
                    tp2 = ptile()
                    nc.tensor.transpose(tp2[0:16, 0:n],
                                        gh3[0:n, c, 128:144],
                                        idn_t[0:n, 0:n])
                    pcopy(ghtb3[:, c, 0:n], tp2[0:16, 0:n])
                g2 = ppool.tile([72, C * 72], F32, tag=f"g2_{f}",
                                name=f"g2_{b}_{f}")
                g23 = g2[:].rearrange("i (c j) -> i c j", c=C)
                for c in range(C):
                    op = ptile()
                    nc.tensor.matmul(op[0:n, 0:n], ghta3[:, c, 0:n],
                                     cda[:, 0:n], start=True, stop=False)
                    nc.tensor.matmul(op[0:n, 0:n], ghtb3[:, c, 0:n],
                                     cdb[:, 0:n], start=False, stop=True)
                    pcopy(g23[0:n, c, 0:n], op[0:n, 0:n])
                out_ap = out_dram.rearrange("c h w -> h c w")
                dma_eng.dma_start(out_ap, g23[0:n, :, 0:n])

            def load_gab(b):
                ga = ppool.tile([128, C * H], F32, tag="gplane_a", bufs=1,
                                name=f"ga{b}")
                gb = ppool.tile([16, C * H], F32, tag="gplane_b", bufs=1,
                                name=f"gb{b}")
                gsrc = gt_d[b].rearrange("c h w -> h c w")
                nc.sync.dma_start(ga[:], gsrc[0:128])
                nc.sync.dma_start(gb[:], gsrc[128:144])
                gar = ppool.tile([128, C * H], F32R, tag="gplane_ar",
                                 bufs=1, name=f"gar{b}")
                gbr = ppool.tile([16, C * H], F32R, tag="gplane_br",
                                 bufs=1, name=f"gbr{b}")
                return ga, gb, gar, gbr

            # ---------- qsq + bias row ----------
            def qsq_bias(b, part_, use_pool):
                """rr bias row 27 = -|q|^2 for cols [lo:hi)."""
                rr = rrs[b]
                qsq = ppool.tile([D, NQ], F32R, tag="qsq", bufs=1,
                                 name=f"qsq{b}_{part_}")
                lo, hi = (0, 2016) if part_ == 0 else (2016, NQ)
                if use_pool:
                    nc.gpsimd.tensor_tensor(qsq[:, lo:hi],
                                            rr[0:D, lo:hi].bitcast(F32),
                                            rr[0:D, lo:hi].bitcast(F32),
                                            op=MUL)
                else:
                    nc.scalar.activation(qsq[:, lo:hi],
                                         rr[0:D, lo:hi].bitcast(F32), SQ)
                for jt in range(lo // 504, hi // 504):
                    bnp = psB.tile([128, CH], F32, tag="psB",
                                   name=f"psbias{b}_{jt}")
                    nc.tensor.matmul(bnp[0:1, 0:504], neg1r[:],
                                     qsq[:, jt * 504:(jt + 1) * 504])
                    sl = rr[KZ:KD, jt * 504:(jt + 1) * 504]
                    if use_pool and jt % 2 == 1:
                        nc.vector.tensor_copy(sl, bnp[0:1, 0:504])
                    else:
                        nc.scalar.copy(sl, bnp[0:1, 0:504])
                if debug and part_ == 1:
                    nc.sync.dma_start(dbg["rr"][b], rrs[b][0:KD, :].bitcast(F32))

            def lr_add(b, h, eng):
                """lr rows 0:27 half h = p1 + p2."""
                sl = slice(h * HNI, (h + 1) * HNI)
                eng.tensor_tensor(lrs[b][0:D, sl], p1ts[b][:, sl],
                                  rrs[b][0:D, sl].bitcast(F32), op=ADD)

            # ---------- main loop tile ----------
            def main_tile(b, t):
                lr, rr = lrs[b], rrs[b]
                rv = rr[0:KD, :].rearrange("p (k two) -> p k two", two=2)
                lrt = lr[:, t * IT:(t + 1) * IT]
                scB = scpool.tile([128, HK], F32, tag="scB",
                                  name=f"scB{b}_{t}")
                for c in range(NCH):
                    pb = psB.tile([128, CH], F32, tag="psB",
                                  name=f"psodd{b}_{t}_{c}")
                    nc.tensor.matmul(pb[:, :], lrt,
                                     rv[:, CH * c:CH * (c + 1), 1])
                    nc.scalar.copy(scB[:, CH * c:CH * (c + 1)], pb[:, :])
                pa = psA.tile([128, NCH, CH], F32, tag="psA",
                              name=f"psA{b}_{t}")
                for c in range(NCH):
                    nc.tensor.matmul(pa[:, c, :], lrt,
                                     rv[:, CH * c:CH * (c + 1), 0])
                flatA = pa[:, :, :].rearrange("p a b -> p (a b)")
                nc.vector._custom_dve(
                    PAIR_OP, out=junk[:], in0=flatA[:, :], in1=scB[:, :],
                    accum_out=idxf[b][:, t:t + 1], imm2=2.0,
                )

            # ---------- tail (per half-image) ----------
            def tail_idx(b, h):
                """PMT permute + wrapped i16 idx for tiles [h*9,(h+1)*9)."""
                wp = psB.tile([128, CH], F32, tag="psB",
                              name=f"pswp{b}_{h}")
                wp3 = wp[0:128, 0:8 * HT].rearrange("p (m t) -> p m t", m=8)
                for m in range(8):
                    nc.tensor.matmul(
                        wp3[:, m, :], pmt_t[:, m * 128:(m + 1) * 128],
                        idxf[b][:, h * HT:(h + 1) * HT],
                    )
                w3 = widxs[b][:].rearrange("p (t m) -> p t m", t=NIT)
                nc.vector.tensor_copy(
                    w3[:, h * HT:(h + 1) * HT, :],
                    wp3[:, :, :].rearrange("p m t -> p t m"))

            def tail_gather(b, h):
                sel = sels[b]
                nc.gpsimd.ap_gather(
                    sel[:, h * HNI:(h + 1) * HNI].rearrange(
                        "p (n d) -> p n d", d=1),
                    rrs[b][0:RG, :].bitcast(F32).rearrange(
                        "p (n d) -> p n d", d=1),
                    widxs[b][0:RG, h * 8 * HT:(h + 1) * 8 * HT],
                    channels=RG, num_elems=NQP, d=1, num_idxs=HNI,
                )
                if debug and h == 1:
                    nc.sync.dma_start(dbg["sel"][b], sels[b][:, :])
                    nc.sync.dma_start(dbg["idxf"][b], idxf[b][:, :])
                    nc.sync.dma_start(dbg["lr"][b], lrs[b][:, :].bitcast(F32))
                    nc.sync.dma_start(dbg["p1t"][b], p1ts[b][:, :])

            def tail_abs(b, h):
                sl = slice(h * HNI, (h + 1) * HNI)
                nc.vector._custom_dve(
                    ABS_OP, out=junk[0:D, 0:HNI], in0=sels[b][0:D, sl],
                    in1=p1ts[b][:, sl],
                    accum_out=part[0:D, 2 * b + h:2 * b + h + 1], imm2=0.0,
                )

            # ================= orchestration =================
            for b in range(B_LOC):
                mk_image_tiles(b)

            # ---- loads, ordered by need (HWDGE serializes ~632ns/DMA) ----
            nc.sync.dma_start(cd4a_f[:], cd4_d[0:128, :])
            nc.sync.dma_start(cd4b_f[:], cd4_d[128:144, :])
            nc.sync.dma_start(cd2a_f[:], cd2_d[0:128, :])
            nc.sync.dma_start(cd2b_f[:], cd2_d[128:144, :])
            nc.vector.tensor_copy(cd4a[:], cd4a_f[:])
            nc.vector.tensor_copy(cd4b[:], cd4b_f[:])
            nc.vector.tensor_copy(cd2a[:], cd2a_f[:])
            nc.vector.tensor_copy(cd2b[:], cd2b_f[:])
            ga0, gb0, gar0, gbr0 = load_gab(0)
            nc.vector.tensor_copy(gar0[:], ga0[:])
            nc.vector.tensor_copy(gbr0[:], gb0[:])

            # pads via Pool SWDGE queue (keeps HWDGE free)
            pad_init(0)
            pad_init(1)

            # ---- image 0 prep ----
            # gt unfold halves on DVE (at-loads on sync right after ga/gb)
            unfold_half(gt_d, 0, 0, rrs[0][0:D, 0:HNI], nc.vector, True,
                        "gt0", nc.scalar)
            unfold_half(gt_d, 0, 1, rrs[0][0:D, HNI:NI], nc.vector, True,
                        "gt0", nc.scalar)
            # x unfold halves on Pool
            unfold_half(x_d, 0, 0, p1ts[0][:, 0:HNI], nc.gpsimd, False,
                        "x0", nc.scalar)
            unfold_half(x_d, 0, 1, p1ts[0][:, HNI:NI], nc.gpsimd, False,
                        "x0", nc.scalar)
            # both images' downsamples in the head (PE+ACT)
            downsample(0, 4, gar0, gbr0, d4_d[0], nc.scalar, nc.scalar)
            downsample(0, 2, gar0, gbr0, d2_d[0], nc.scalar, nc.scalar)
            ga1, gb1, gar1, gbr1 = load_gab(1)
            nc.vector.tensor_copy(gar1[:], ga1[:])
            nc.vector.tensor_copy(gbr1[:], gb1[:])
            downsample(1, 4, gar1, gbr1, d4_d[1], nc.scalar, nc.scalar)
            downsample(1, 2, gar1, gbr1, d2_d[1], nc.scalar, nc.scalar)
            # image-0 small unfolds (DVE) + qsq/bias (ACT+PE)
            unfold_small(d4_d, 0, 4, rrs[0][0:D, NI + 576:NQ], nc.vector,
                         "d40", nc.scalar)
            unfold_small(d2_d, 0, 2, rrs[0][0:D, NI:NI + 576], nc.vector,
                         "d20", nc.scalar)
            qsq_bias(0, 0, use_pool=False)
            qsq_bias(0, 1, use_pool=False)
            # lr adds (DVE)
            lr_add(0, 0, nc.vector)
            lr_add(0, 1, nc.vector)


            # ---- main(0) with image-1 prep interleaved ----
            for t in range(NIT):
                main_tile(0, t)
                if t == 0:
                    unfold_half(gt_d, 1, 0, rrs[1][0:D, 0:HNI],
                                nc.gpsimd, True, "gt1", nc.sync)
                elif t == 2:
                    unfold_half(gt_d, 1, 1, rrs[1][0:D, HNI:NI],
                                nc.gpsimd, True, "gt1", nc.sync)
                elif t == 4:
                    unfold_half(x_d, 1, 0, p1ts[1][:, 0:HNI],
                                nc.gpsimd, False, "x1", nc.sync)
                elif t == 6:
                    unfold_half(x_d, 1, 1, p1ts[1][:, HNI:NI],
                                nc.gpsimd, False, "x1", nc.sync)
                elif t == 7:
                    nc.sync.dma_start(pmt_t[:], pmt_d[:])
                elif t == 9:
                    unfold_small(d4_d, 1, 4, rrs[1][0:D, NI + 576:NQ],
                                 nc.vector, "d41", nc.sync)
                elif t == 10:
                    unfold_small(d2_d, 1, 2, rrs[1][0:D, NI:NI + 576],
                                 nc.vector, "d21", nc.sync)
                elif t == 11:
                    qsq_bias(1, 0, use_pool=True)
                elif t == 12:
                    lr_add(1, 0, nc.vector)
                elif t == 14:
                    qsq_bias(1, 1, use_pool=True)
                elif t == 15:
                    lr_add(1, 1, nc.vector)

            # ---- main(1) with image-0 tail interleaved ----
            for t in range(NIT):
                main_tile(1, t)
                if t == 0:
                    tail_idx(0, 0)
                elif t == 1:
                    tail_gather(0, 0)
                elif t == 3:
                    tail_idx(0, 1)
                    tail_abs(0, 0)
                elif t == 4:
                    tail_gather(0, 1)
                elif t == 6:
                    tail_abs(0, 1)
                elif t == 9:
                    tail_idx(1, 0)
                elif t == 10:
                    tail_gather(1, 0)
                elif t == 12:
                    tail_abs(1, 0)

            # ---- image-1 second-half tail ----
            tail_idx(1, 1)
            tail_gather(1, 1)
            tail_abs(1, 1)

            nc.sync.dma_start(loss_d[:], part[0:D, :])

    nc.compile()
    return nc


_NC_CACHE = None


def _get_nc():
    global _NC_CACHE
    if _NC_CACHE is None:
        _NC_CACHE = build_nc()
    return _NC_CACHE


def kernel(x: np.ndarray, gt: np.ndarray, _trace=False, _debug=False):
    x = np.ascontiguousarray(np.asarray(x, dtype=np.float32))
    gt = np.ascontiguousarray(np.asarray(gt, dtype=np.float32))
    consts = make_consts()
    nc = build_nc(debug=True) if _debug else _get_nc()
    in_maps = []
    for c in range(NCORES):
        m = {"x": x[c * B_LOC:(c + 1) * B_LOC],
             "gt": gt[c * B_LOC:(c + 1) * B_LOC]}
        m.update(consts)
        in_maps.append(m)
    res = run_bass_kernel_spmd(
        nc, in_maps, core_ids=list(range(NCORES)), trace=_trace,
        trace_cores=[0] if _trace else None,
    )
    total = sum(float(r["loss"].sum()) for r in res.results)
    out = np.asarray(np.float32(total / (B_FULL * NI * D)))
    if _trace or _debug:
        return out, res
    return out


# revision 12
# speedup vs baseline: 1.0372x; 1.0372x over previous
"""BestBuddyLoss Trainium2 kernel v3 (8-core data parallel).

Per image: q = [unfold(gt) | unfold(down2(gt)) | unfold(down4(gt))] padded to
3072 cols; argmin_j score(i,j) == argmax_j <p1_i+p2_i, q_j> - |q_j|^2, done as
K=28 f32r matmuls (27 data rows + bias row 27).  Logical column j = 2k + b:
even columns stream through stride-2 matmuls into PSUM (3 banks flat,
[128,1536]); odd columns land in transit PSUM banks and are copied to SBUF by
ACT.  One 2-stream custom DVE op per i-tile scans (A_k, B_k) pairs and emits
argmax j = 2*Idx + (A<B) via running-max records.  Tail: PMT permute ->
wrapped i16 idx -> ap_gather q[j*] -> fused |p1 - q*| sum on DVE (custom op,
accum=ADD), pipelined in half-image chunks under the other image's main loop.

Schedule: image-0 prep split across DVE (gt shuffles, adds) / ACT (qsq, bias
copies, downsample copies) / Pool (x shuffle, memsets) / PE, first scan at
~14us.  Image-1 prep runs on Pool + DVE slack interleaved into main(0)'s
emission; its bias matmuls borrow psB slots.
"""

import sys

sys.path.insert(0, "/opt/trn_rl_repo")

import numpy as np

import concourse.bacc as bacc
import concourse.mybir as mybir
import concourse.tile as tile
from concourse.bass_utils import run_bass_kernel_spmd

# ---------------- problem constants (hardcoded) ----------------
B_FULL = 16
NCORES = 8
B_LOC = B_FULL // NCORES       # images per core
C, H, W = 3, 144, 144
G = 48                         # patch grid (144/3)
NI = G * G                     # 2304 query patches
D = 27                         # C*3*3
NQ = NI + (G // 2) ** 2 + (G // 4) ** 2  # 3024
NQP = 3072                     # padded j space (6*512)
KD = 33                        # contraction: 27 data + 5 zero + bias row 32
KZ = 32                        # bias row partition (32-aligned for engine APs)
RG = 32                        # gather channels
IT = 128
NIT = NI // IT                 # 18
HK = NQP // 2                  # 1536 pair count per tile
CH = 512
NCH = HK // CH                 # 3 chunks per half
HNI = NI // 2                  # 1152 per tail half
HT = NIT // 2                  # 9 tiles per tail half
PADBIAS = -1.0e30
CUBIC_W = np.array([-0.09375, 0.59375, 0.59375, -0.09375], dtype=np.float32)

F32 = mybir.dt.float32
F32R = mybir.dt.float32r
I16 = mybir.dt.int16
ADD = mybir.AluOpType.add
SUB = mybir.AluOpType.subtract
MUL = mybir.AluOpType.mult
SQ = mybir.ActivationFunctionType.Square

# ---------------- custom DVE ops ------------------
from concourse.dve_spec import (
    Spec, Src0, Src1, C2, Zero, scan, AluOp, maxx, lower,
)
from concourse.dve_uop import DveOpSpec
import concourse.dve_ops as dve_ops
from concourse.dve_ops import DveOp


def _pair_argmax_ref(in0, in1, c0, c1, c2):
    m = np.maximum(in0, in1)
    run = np.maximum.accumulate(m, axis=-1)
    n = in0.shape[-1]
    j = np.float32(c2) * np.arange(n, dtype=np.float32)[None, :] + (in0 < in1)
    out = (m >= run) * j
    acc = out.reshape(out.shape[0], -1).max(axis=-1, keepdims=True)
    return out.astype(np.float32), acc.astype(np.float32)


def _absdiff_acc_ref(in0, in1, c0, c1, c2):
    out = np.abs(in0 - in1).astype(np.float32)
    acc = out.reshape(out.shape[0], -1).sum(axis=-1, keepdims=True)
    return out, acc.astype(np.float32)


def _register_op(name, spec):
    if name in dve_ops._SUB_OPCODE_FOR_NAME:
        return next(op for op in dve_ops.OPS if op.name == name)
    opcode = dve_ops._CUSTOM_DVE_ROW_BASE + len(dve_ops.OPS)
    shas = {v: DveOpSpec(name=name, opcode=opcode, uops=lower(spec, ver=v),
                         rd1_en=True).sha(v) for v in ("v3", "v4")}
    op = DveOp(name, spec, subdim=False, uops_sha=shas)
    dve_ops.OPS.append(op)
    dve_ops._SUB_OPCODE_FOR_NAME[name] = opcode
    dve_ops.CUSTOM_DVE_SPECS[name] = spec
    return op


def _make_pair_op():
    m = maxx(Src0, Src1)
    two_idx = scan(AluOp.ADD, C2, init=Zero - C2)   # 0, 2, 4, ... (C2=2.0)
    j = two_idx + (Src0 < Src1)
    body = (m >= scan(AluOp.MAX, m)) * j
    return _register_op(
        "ANT_PAIR_ARGMAX",
        Spec(body=body, accum=maxx, reference=_pair_argmax_ref))


def _make_absdiff_op():
    body = maxx(Src0 - Src1, Src1 - Src0)
    return _register_op(
        "ANT_ABSDIFF_ACC",
        Spec(body=body, accum=AluOp.ADD, reference=_absdiff_acc_ref))


PAIR_OP = _make_pair_op()
ABS_OP = _make_absdiff_op()

# ---------------- host-side constants ---------------------------------


def _down_matrix(n, f):
    """M[h, i]: out[i] = sum_h M[h, i] * in[h]  (torch bicubic, offset t=.5)."""
    out_n = n // f
    M = np.zeros((n, out_n), dtype=np.float32)
    for i in range(out_n):
        base = f * i + (f // 2 - 1)
        for a in range(4):
            h = min(max(base + a - 1, 0), n - 1)
            M[h, i] += CUBIC_W[a]
    return M


def _perm_matrices():
    """PMT[:, m*128 + r]: one-hot at row (m*16 + r%16) -> out_m = Pm @ v."""
    P = np.zeros((128, 8 * 128), dtype=np.float32)
    for m in range(8):
        for r in range(128):
            P[m * 16 + r % 16, m * 128 + r] = 1.0
    return P


def make_consts():
    return {
        "cd2": np.ascontiguousarray(_down_matrix(H, 2)),  # [144, 72]
        "cd4": np.ascontiguousarray(_down_matrix(H, 4)),  # [144, 36]
        "idn": np.eye(128, dtype=np.float32),
        "pmt": _perm_matrices(),
        "neg1": np.full((D, 1), -1.0, dtype=np.float32),
        "padq": _padq(),
        "padl": _padl(),
    }


def _padq():
    p = np.zeros((KD - D, NQP), dtype=np.float32)
    p[KZ - D, NQ:NQP] = PADBIAS
    return p


def _padl():
    p = np.zeros((KD - D, NI), dtype=np.float32)
    p[KZ - D, :] = 1.0
    return p


# ---------------- kernel construction ---------------------------------


def build_nc(debug=False):
    nc = bacc.Bacc("TRN2", target_bir_lowering=False)

    x_d = nc.dram_tensor("x", [B_LOC, C, H, W], F32, kind="ExternalInput")
    gt_d = nc.dram_tensor("gt", [B_LOC, C, H, W], F32, kind="ExternalInput")
    cd2_d = nc.dram_tensor("cd2", [H, 72], F32, kind="ExternalInput")
    cd4_d = nc.dram_tensor("cd4", [H, 36], F32, kind="ExternalInput")
    idn_d = nc.dram_tensor("idn", [128, 128], F32, kind="ExternalInput")
    pmt_d = nc.dram_tensor("pmt", [128, 8 * 128], F32, kind="ExternalInput")
    neg1_d = nc.dram_tensor("neg1", [D, 1], F32, kind="ExternalInput")
    padq_d = nc.dram_tensor("padq", [KD - D, NQP], F32R, kind="ExternalInput")
    padl_d = nc.dram_tensor("padl", [KD - D, NI], F32R, kind="ExternalInput")
    d2_d = nc.dram_tensor("scr_d2", [B_LOC, C, 72, 72], F32, kind="Internal")
    d4_d = nc.dram_tensor("scr_d4", [B_LOC, C, 36, 36], F32, kind="Internal")
    loss_d = nc.dram_tensor("loss", [D, 4], F32, kind="ExternalOutput")
    dbg = {}
    if debug:
        dbg["rr"] = nc.dram_tensor("dbg_rr", [B_LOC, KD, NQP], F32, kind="ExternalOutput")
        dbg["lr"] = nc.dram_tensor("dbg_lr", [B_LOC, KD, NI], F32, kind="ExternalOutput")
        dbg["p1t"] = nc.dram_tensor("dbg_p1t", [B_LOC, D, NI], F32, kind="ExternalOutput")
        dbg["idxf"] = nc.dram_tensor("dbg_idxf", [B_LOC, 128, NIT], F32, kind="ExternalOutput")
        dbg["sel"] = nc.dram_tensor("dbg_sel", [B_LOC, 32, NI], F32, kind="ExternalOutput")

    with tile.TileContext(nc) as tc:
        with (
            tc.tile_pool(name="consts", bufs=1) as cpool,
            tc.tile_pool(name="stageA", bufs=2) as apool,
            tc.tile_pool(name="stageB", bufs=2) as bpool,
            tc.tile_pool(name="stageD", bufs=1) as dapool,
            tc.tile_pool(name="prep", bufs=2) as ppool,
            tc.tile_pool(name="persist", bufs=2) as spool,
            tc.tile_pool(name="scoreB", bufs=2) as scpool,
            tc.tile_pool(name="small", bufs=2) as smpool,
            tc.tile_pool(name="psA", bufs=2, space="PSUM") as psA,
            tc.tile_pool(name="psB", bufs=2, space="PSUM") as psB,
        ):
            # ---------- consts ----------
            cd2a_f = cpool.tile([128, 72], F32, tag="cd2af")
            cd2b_f = cpool.tile([16, 72], F32, tag="cd2bf")
            cd4a_f = cpool.tile([128, 36], F32, tag="cd4af")
            cd4b_f = cpool.tile([16, 36], F32, tag="cd4bf")
            cd2a = cpool.tile([128, 72], F32R, tag="cd2a")
            cd2b = cpool.tile([16, 72], F32R, tag="cd2b")
            cd4a = cpool.tile([128, 36], F32R, tag="cd4a")
            cd4b = cpool.tile([16, 36], F32R, tag="cd4b")
            idn_t = cpool.tile([128, 128], F32, tag="idn")
            pmt_t = cpool.tile([128, 8 * 128], F32, tag="pmt")
            neg1_t = cpool.tile([D, 1], F32, tag="neg1")
            neg1r = cpool.tile([D, 1], F32R, tag="neg1r")
            nc.sync.dma_start(cd2a_f[:], cd2_d[0:128, :])
            nc.sync.dma_start(cd2b_f[:], cd2_d[128:144, :])
            nc.sync.dma_start(cd4a_f[:], cd4_d[0:128, :])
            nc.sync.dma_start(cd4b_f[:], cd4_d[128:144, :])
            nc.sync.dma_start(idn_t[:], idn_d[:])
            nc.sync.dma_start(pmt_t[:], pmt_d[:])
            nc.sync.dma_start(neg1_t[:], neg1_d[:])
            nc.vector.tensor_copy(cd2a[:], cd2a_f[:])
            nc.vector.tensor_copy(cd2b[:], cd2b_f[:])
            nc.vector.tensor_copy(cd4a[:], cd4a_f[:])
            nc.vector.tensor_copy(cd4b[:], cd4b_f[:])
            nc.vector.tensor_copy(neg1r[:], neg1_t[:])

            junk = cpool.tile([128, HK], F32, tag="junk")
            part = cpool.tile([D, 4], F32, tag="part")

            # ---------- persistent per-image tiles (bufs=2) ----------
            rrs, lrs, p1ts, idxf, widxs, sels = ({} for _ in range(6))

            def mk_image_tiles(b):
                rrs[b] = spool.tile([KD, NQP], F32R, tag="rr", name=f"rr{b}")
                lrs[b] = spool.tile([KD, NI], F32R, tag="lr", name=f"lr{b}")
                p1ts[b] = spool.tile([D, NI], F32, tag="p1t", name=f"p1t{b}")
                idxf[b] = smpool.tile([128, NIT], F32, tag="idxf",
                                      name=f"idxf{b}")
                widxs[b] = smpool.tile([128, 8 * NIT], I16, tag="widx",
                                       name=f"widx{b}")
                sels[b] = smpool.tile([RG, NI], F32, tag="sel", bufs=1,
                                      name=f"sel{b}")

            def pad_init(b):
                rr, lr = rrs[b], lrs[b]
                # pad columns of the data rows (32-aligned partition start)
                nc.gpsimd.memset(rr[0:KZ, NQ:NQP].bitcast(F32), 0.0)
                # rows 27:33: zeros + bias/ones rows via one DMA each
                nc.gpsimd.dma_start(rr[D:KD, :], padq_d[:])
                nc.gpsimd.dma_start(lr[D:KD, :], padl_d[:])

            # ---------- unfold helpers ----------
            def shuffle(eng, dst_v, src_v):
                if eng is nc.scalar:
                    eng.copy(dst_v, src_v)
                else:
                    eng.tensor_copy(dst_v, src_v)

            def unfold_half(td, b, hf, dst_ap, eng, round_f32r, nm,
                            dma_eng):
                """one half of dram [C,144,144] -> dst [27, 1152]."""
                gh = G // 2
                szh = 3 * gh * G
                at = apool.tile([9, szh], F32, tag="A", name=f"at_{nm}{hf}")
                for c in range(C):
                    src = td[b, c].rearrange(
                        "(gi r) w -> r gi w", r=3)[:, hf * gh:(hf + 1) * gh]
                    nc.sync.dma_start(at[3 * c:3 * c + 3, :], src)
                bt = bpool.tile([9, szh], F32R if round_f32r else F32,
                                tag="B", name=f"bt_{nm}{hf}")
                rearr = at[:, :].rearrange(
                    "p (gi gj s) -> p s gi gj", gi=gh, gj=G, s=3)
                bt_v = bt[:, :].rearrange(
                    "p (s gi gj) -> p s gi gj", s=3, gi=gh)
                shuffle(eng, bt_v, rearr)
                if round_f32r:
                    dma_eng.dma_start(dst_ap.bitcast(F32),
                                      bt[:, :].bitcast(F32))
                else:
                    dma_eng.dma_start(dst_ap, bt[:, :])

            def unfold_small(dsc_d, b, f, dst_ap, eng, nm, dma_eng):
                """downsampled DRAM scratch [C, n, n] -> dst [27, gs*gs]."""
                n = H // f
                gs = n // 3
                sz = 3 * gs * gs
                dat = dapool.tile([9, 3 * 24 * 24], F32, tag="DA",
                                  name=f"dat_{nm}")
                for c in range(C):
                    src = dsc_d[b, c].rearrange("(gi r) w -> r gi w", r=3)
                    nc.sync.dma_start(
                        dat[3 * c:3 * c + 3, 0:gs * n].rearrange(
                            "p (gi w) -> p gi w", gi=gs), src)
                bt = bpool.tile([9, 3 * (G // 2) * G], F32R, tag="B",
                                name=f"bt_{nm}")
                rearr = dat[:, 0:sz].rearrange(
                    "p (gi gj s) -> p s gi gj", gi=gs, gj=gs, s=3)
                bt_v = bt[:, 0:sz].rearrange(
                    "p (s gi gj) -> p s gi gj", s=3, gi=gs)
                shuffle(eng, bt_v, rearr)
                dma_eng.dma_start(dst_ap.bitcast(F32),
                                  bt[:, 0:sz].bitcast(F32))

            # ---------- downsample (all f32r) ----------
            def downsample(b, f, gar, gbr, out_dram, ceng, dma_eng):
                """gt[b] (f32r [128/16, C*H]) --bicubic/f--> out_dram."""
                n = H // f
                cda = cd2a if f == 2 else cd4a
                cdb = cd2b if f == 2 else cd4b

                cnt = [0]

                def ptile():
                    cnt[0] += 1
                    return psB.tile([128, CH], F32, tag="psB",
                                    name=f"psds{b}_{f}_{cnt[0]}")

                def pcopy(dst, srcp):
                    if ceng is nc.scalar:
                        ceng.copy(dst, srcp)
                    else:
                        ceng.tensor_copy(dst, srcp)

                ghp = ptile()
                nc.tensor.matmul(ghp[0:n, 0:C * H], cda[:, 0:n], gar[:],
                                 start=True, stop=False)
                nc.tensor.matmul(ghp[0:n, 0:C * H], cdb[:, 0:n], gbr[:],
                                 start=False, stop=True)
                gh = ppool.tile([72, C * H], F32, tag="gh", bufs=1,
                                name=f"gh{b}_{f}")
                pcopy(gh[0:n, :], ghp[0:n, 0:C * H])
                gh3 = gh[:].rearrange("i (c w) -> i c w", c=C)
                ghta = ppool.tile([128, C * 72], F32R, tag="ghta",
                                  name=f"ghta{b}_{f}")
                ghtb = ppool.tile([16, C * 72], F32R, tag="ghtb",
                                  name=f"ghtb{b}_{f}")
                ghta3 = ghta[:].rearrange("w (c i) -> w c i", c=C)
                ghtb3 = ghtb[:].rearrange("w (c i) -> w c i", c=C)
                for c in range(C):
                    tp = ptile()
                    nc.tensor.transpose(tp[0:128, 0:n], gh3[0:n, c, 0:128],
                                        idn_t[0:n, 0:n])
                    pcopy(ghta3[:, c, 0:n], tp[0:128, 0:n])
                    tp2 = ptile()
                    nc.tensor.transpose(tp2[0:16, 0:n],
                                        gh3[0:n, c, 128:144],
                                        idn_t[0:n, 0:n])
                    pcopy(ghtb3[:, c, 0:n], tp2[0:16, 0:n])
                g2 = ppool.tile([72, C * 72], F32, tag=f"g2_{f}",
                                name=f"g2_{b}_{f}")
                g23 = g2[:].rearrange("i (c j) -> i c j", c=C)
                for c in range(C):
                    op = ptile()
                    nc.tensor.matmul(op[0:n, 0:n], ghta3[:, c, 0:n],
                                     cda[:, 0:n], start=True, stop=False)
                    nc.tensor.matmul(op[0:n, 0:n], ghtb3[:, c, 0:n],
                                     cdb[:, 0:n], start=False, stop=True)
                    pcopy(g23[0:n, c, 0:n], op[0:n, 0:n])
                out_ap = out_dram.rearrange("c h w -> h c w")
                dma_eng.dma_start(out_ap, g23[0:n, :, 0:n])

            def load_gab(b):
                ga = ppool.tile([128, C * H], F32, tag="gplane_a", bufs=1,
                                name=f"ga{b}")
                gb = ppool.tile([16, C * H], F32, tag="gplane_b", bufs=1,
                                name=f"gb{b}")
                gsrc = gt_d[b].rearrange("c h w -> h c w")
                nc.sync.dma_start(ga[:], gsrc[0:128])
                nc.sync.dma_start(gb[:], gsrc[128:144])
                gar = ppool.tile([128, C * H], F32R, tag="gplane_ar",
                                 bufs=1, name=f"gar{b}")
                gbr = ppool.tile([16, C * H], F32R, tag="gplane_br",
                                 bufs=1, name=f"gbr{b}")
                return ga, gb, gar, gbr

            # ---------- qsq + bias row ----------
            def qsq_bias(b, part_, use_pool):
                """rr bias row 27 = -|q|^2 for cols [lo:hi)."""
                rr = rrs[b]
                qsq = ppool.tile([D, NQ], F32R, tag="qsq", bufs=1,
                                 name=f"qsq{b}_{part_}")
                lo, hi = (0, 2016) if part_ == 0 else (2016, NQ)
                if use_pool:
                    nc.gpsimd.tensor_tensor(qsq[:, lo:hi],
                                            rr[0:D, lo:hi].bitcast(F32),
                                            rr[0:D, lo:hi].bitcast(F32),
                                            op=MUL)
                else:
                    nc.scalar.activation(qsq[:, lo:hi],
                                         rr[0:D, lo:hi].bitcast(F32), SQ)
                for jt in range(lo // 504, hi // 504):
                    bnp = psB.tile([128, CH], F32, tag="psB",
                                   name=f"psbias{b}_{jt}")
                    nc.tensor.matmul(bnp[0:1, 0:504], neg1r[:],
                                     qsq[:, jt * 504:(jt + 1) * 504])
                    sl = rr[KZ:KD, jt * 504:(jt + 1) * 504]
                    if use_pool and jt % 2 == 1:
                        nc.vector.tensor_copy(sl, bnp[0:1, 0:504])
                    else:
                        nc.scalar.copy(sl, bnp[0:1, 0:504])
                if debug and part_ == 1:
                    nc.sync.dma_start(dbg["rr"][b], rrs[b][0:KD, :].bitcast(F32))

            def lr_add(b, h, eng):
                """lr rows 0:27 half h = p1 + p2."""
                sl = slice(h * HNI, (h + 1) * HNI)
                eng.tensor_tensor(lrs[b][0:D, sl], p1ts[b][:, sl],
                                  rrs[b][0:D, sl].bitcast(F32), op=ADD)

            # ---------- main loop tile ----------
            def main_tile(b, t):
                lr, rr = lrs[b], rrs[b]
                rv = rr[0:KD, :].rearrange("p (k two) -> p k two", two=2)
                lrt = lr[:, t * IT:(t + 1) * IT]
                scB = scpool.tile([128, HK], F32, tag="scB",
                                  name=f"scB{b}_{t}")
                for c in range(NCH):
                    pb = psB.tile([128, CH], F32, tag="psB",
                                  name=f"psodd{b}_{t}_{c}")
                    nc.tensor.matmul(pb[:, :], lrt,
                                     rv[:, CH * c:CH * (c + 1), 1])
                    nc.scalar.copy(scB[:, CH * c:CH * (c + 1)], pb[:, :])
                pa = psA.tile([128, NCH, CH], F32, tag="psA",
                              name=f"psA{b}_{t}")
                for c in range(NCH):
                    nc.tensor.matmul(pa[:, c, :], lrt,
                                     rv[:, CH * c:CH * (c + 1), 0])
                flatA = pa[:, :, :].rearrange("p a b -> p (a b)")
                nc.vector._custom_dve(
                    PAIR_OP, out=junk[:], in0=flatA[:, :], in1=scB[:, :],
                    accum_out=idxf[b][:, t:t + 1], imm2=2.0,
                )

            # ---------- tail (per half-image) ----------
            def tail_idx(b, h):
                """PMT permute + wrapped i16 idx for tiles [h*9,(h+1)*9)."""
                wp = psB.tile([128, CH], F32, tag="psB",
                              name=f"pswp{b}_{h}")
                wp3 = wp[0:128, 0:8 * HT].rearrange("p (m t) -> p m t", m=8)
                for m in range(8):
                    nc.tensor.matmul(
                        wp3[:, m, :], pmt_t[:, m * 128:(m + 1) * 128],
                        idxf[b][:, h * HT:(h + 1) * HT],
                    )
                w3 = widxs[b][:].rearrange("p (t m) -> p t m", t=NIT)
                nc.vector.tensor_copy(
                    w3[:, h * HT:(h + 1) * HT, :],
                    wp3[:, :, :].rearrange("p m t -> p t m"))

            def tail_gather(b, h):
                sel = sels[b]
                nc.gpsimd.ap_gather(
                    sel[:, h * HNI:(h + 1) * HNI].rearrange(
                        "p (n d) -> p n d", d=1),
                    rrs[b][0:RG, :].bitcast(F32).rearrange(
                        "p (n d) -> p n d", d=1),
                    widxs[b][0:RG, h * 8 * HT:(h + 1) * 8 * HT],
                    channels=RG, num_elems=NQP, d=1, num_idxs=HNI,
                )
                if debug and h == 1:
                    nc.sync.dma_start(dbg["sel"][b], sels[b][:, :])
                    nc.sync.dma_start(dbg["idxf"][b], idxf[b][:, :])
                    nc.sync.dma_start(dbg["lr"][b], lrs[b][:, :].bitcast(F32))
                    nc.sync.dma_start(dbg["p1t"][b], p1ts[b][:, :])

            def tail_abs(b, h):
                sl = slice(h * HNI, (h + 1) * HNI)
                nc.vector._custom_dve(
                    ABS_OP, out=junk[0:D, 0:HNI], in0=sels[b][0:D, sl],
                    in1=p1ts[b][:, sl],
                    accum_out=part[0:D, 2 * b + h:2 * b + h + 1], imm2=0.0,
                )

            # ================= orchestration =================
            for b in range(B_LOC):
                mk_image_tiles(b)

            # ---- image-0 at loads first (HWDGE ~632ns/DMA serializes) ----
            # gt/x at-loads are emitted inside unfold_half below; emit the
            # unfolds first so their loads get the earliest HWDGE slots.
            unfold_half(gt_d, 0, 0, rrs[0][0:D, 0:HNI], nc.vector, True,
                        "gt0", nc.gpsimd)
            unfold_half(gt_d, 0, 1, rrs[0][0:D, HNI:NI], nc.vector, True,
                        "gt0", nc.gpsimd)
            unfold_half(x_d, 0, 0, p1ts[0][:, 0:HNI], nc.gpsimd, False,
                        "x0", nc.gpsimd)
            ga0, gb0, gar0, gbr0 = load_gab(0)
            nc.sync.dma_start(cd4a_f[:], cd4_d[0:128, :])
            nc.sync.dma_start(cd4b_f[:], cd4_d[128:144, :])
            nc.sync.dma_start(cd2a_f[:], cd2_d[0:128, :])
            nc.sync.dma_start(cd2b_f[:], cd2_d[128:144, :])
            nc.vector.tensor_copy(gar0[:], ga0[:])
            nc.vector.tensor_copy(gbr0[:], gb0[:])
            nc.vector.tensor_copy(cd4a[:], cd4a_f[:])
            nc.vector.tensor_copy(cd4b[:], cd4b_f[:])
            nc.vector.tensor_copy(cd2a[:], cd2a_f[:])
            nc.vector.tensor_copy(cd2b[:], cd2b_f[:])
            unfold_half(x_d, 0, 1, p1ts[0][:, HNI:NI], nc.gpsimd, False,
                        "x0", nc.gpsimd)
            ga1, gb1, gar1, gbr1 = load_gab(1)

            # pads via Pool SWDGE queue (keeps HWDGE free)
            pad_init(0)
            pad_init(1)

            # both images' downsamples in the head (PE+ACT)
            downsample(0, 4, gar0, gbr0, d4_d[0], nc.scalar, nc.scalar)
            downsample(0, 2, gar0, gbr0, d2_d[0], nc.scalar, nc.scalar)
            # image-0 small unfolds (DVE) + qsq/bias part 0 (ACT+PE)
            unfold_small(d4_d, 0, 4, rrs[0][0:D, NI + 576:NQ], nc.vector,
                         "d40", nc.scalar)
            qsq_bias(0, 0, use_pool=False)
            unfold_small(d2_d, 0, 2, rrs[0][0:D, NI:NI + 576], nc.vector,
                         "d20", nc.scalar)
            # image-1 downsample after image-0's critical ACT work
            nc.vector.tensor_copy(gar1[:], ga1[:])
            nc.vector.tensor_copy(gbr1[:], gb1[:])
            downsample(1, 4, gar1, gbr1, d4_d[1], nc.scalar, nc.scalar)
            qsq_bias(0, 1, use_pool=False)
            downsample(1, 2, gar1, gbr1, d2_d[1], nc.scalar, nc.scalar)
            # lr adds (DVE)
            lr_add(0, 0, nc.vector)
            lr_add(0, 1, nc.vector)

            # ---- main(0) with image-1 prep interleaved ----
            for t in range(NIT):
                main_tile(0, t)
                if t == 0:
                    # all image-1 loads: dependency-free, batch on sync
                    unfold_half(gt_d, 1, 0, rrs[1][0:D, 0:HNI],
                                nc.gpsimd, True, "gt1", nc.gpsimd)
                    unfold_half(gt_d, 1, 1, rrs[1][0:D, HNI:NI],
                                nc.gpsimd, True, "gt1", nc.gpsimd)
                elif t == 1:
                    unfold_half(x_d, 1, 0, p1ts[1][:, 0:HNI],
                                nc.gpsimd, False, "x1", nc.gpsimd)
                    unfold_half(x_d, 1, 1, p1ts[1][:, HNI:NI],
                                nc.gpsimd, False, "x1", nc.gpsimd)
                elif t == 2:
                    unfold_small(d4_d, 1, 4, rrs[1][0:D, NI + 576:NQ],
                                 nc.vector, "d41", nc.gpsimd)
                elif t == 3:
                    unfold_small(d2_d, 1, 2, rrs[1][0:D, NI:NI + 576],
                                 nc.vector, "d21", nc.gpsimd)
                elif t == 4:
                    nc.sync.dma_start(pmt_t[:], pmt_d[:])
                elif t == 7:
                    qsq_bias(1, 0, use_pool=True)
                elif t == 8:
                    lr_add(1, 0, nc.vector)
                elif t == 12:
                    qsq_bias(1, 1, use_pool=True)
                elif t == 13:
                    lr_add(1, 1, nc.vector)

            # ---- main(1) with image-0 tail interleaved ----
            for t in range(NIT):
                main_tile(1, t)
                if t == 0:
                    tail_idx(0, 0)
                elif t == 1:
                    tail_gather(0, 0)
                elif t == 3:
                    tail_idx(0, 1)
                    tail_abs(0, 0)
                elif t == 4:
                    tail_gather(0, 1)
                elif t == 6:
                    tail_abs(0, 1)
                elif t == 9:
                    tail_idx(1, 0)
                elif t == 10:
                    tail_gather(1, 0)
                elif t == 12:
                    tail_abs(1, 0)

            # ---- image-1 second-half tail ----
            tail_idx(1, 1)
            tail_gather(1, 1)
            tail_abs(1, 1)

            nc.sync.dma_start(loss_d[:], part[0:D, :])

    nc.compile()
    return nc


_NC_CACHE = None


def _get_nc():
    global _NC_CACHE
    if _NC_CACHE is None:
        _NC_CACHE = build_nc()
    return _NC_CACHE


def kernel(x: np.ndarray, gt: np.ndarray, _trace=False, _debug=False):
    x = np.ascontiguousarray(np.asarray(x, dtype=np.float32))
    gt = np.ascontiguousarray(np.asarray(gt, dtype=np.float32))
    consts = make_consts()
    nc = build_nc(debug=True) if _debug else _get_nc()
    in_maps = []
    for c in range(NCORES):
        m = {"x": x[c * B_LOC:(c + 1) * B_LOC],
             "gt": gt[c * B_LOC:(c + 1) * B_LOC]}
        m.update(consts)
        in_maps.append(m)
    res = run_bass_kernel_spmd(
        nc, in_maps, core_ids=list(range(NCORES)), trace=_trace,
        trace_cores=[0] if _trace else None,
    )
    total = sum(float(r["loss"].sum()) for r in res.results)
    out = np.asarray(np.float32(total / (B_FULL * NI * D)))
    if _trace or _debug:
        return out, res
    return out
